# revision 24
# baseline (speedup 1.0000x reference)
"""MemoryUnit kernel for Trainium2 (8 NeuronCores, data-parallel over batch).

Computes, for x [4096,4096] and weight [2000,4096] (fp32):
  logits = cos_sim(x, weight)           # [B, M]
  mem_weight = renorm(shrink(softmax(logits)))
  output = mem_weight @ weight          # [B, D]
Returns (output, mem_weight) like the reference.

Sharding: batch split 512 rows/core; weight replicated.
Per-core pipeline (v3):
  - x rows normalized, split into fp16 hi/lo pair (lo scaled by 2048),
    PE-transposed to x^T halves [D_p, B_f]
  - mm1 as three fp16 matmul passes (hi*hi + (hi*lo + lo*hi)/2048) with
    fp32 PSUM accumulation — numerically at the fp32 noise floor;
    weight tiles split+transposed on the fly, software-pipelined one
    tile ahead; weight-hi also written to a DRAM scratch for mm2;
    1/||w|| folded into the PSUM->SBUF combine
  - softmax with max-subtraction, two-pass exp (no full E buffer);
    row-sum computed as M + sum(exp-1) for fp32-exact accumulation;
    hard shrinkage + L1 renorm (all fp32)
  - mm2 (fp16 operands, fp32 accum): out = mem_weight @ weight_hi with
    mem_weight^T built via PE transposes and weight_hi streamed back
    from the DRAM scratch in four pipelined quarter-tiles
"""

import numpy as np

B_FULL, D, M = 4096, 4096, 2000
NCORES = 8
BS = B_FULL // NCORES          # 512 batch rows per core
NBT = BS // 128                # 4 batch partition-tiles
NKC = D // 128                 # 32 contraction chunks of 128
NMT = 16
MT_SIZES = [128] * 15 + [80]   # 2000 = 15*128 + 80
MT_OFFS = [sum(MT_SIZES[:i]) for i in range(NMT)]
SHRINK_T = 1.0 / M
SHRINK_EPS = 1e-12
CH = 500                       # softmax free-dim chunk (4 * 500 = 2000)
NCH = M // CH
LO_SCALE = 2048.0              # fp16 residual scaling (2^11)

_cache = {}


def _build_nc():
    import concourse.mybir as mybir
    from concourse import bacc
    from concourse.masks import make_identity
    from concourse.tile import TileContext

    f32 = mybir.dt.float32
    f16 = mybir.dt.float16
    AF = mybir.ActivationFunctionType
    ALU = mybir.AluOpType
    AX = mybir.AxisListType

    nc = bacc.Bacc("TRN2", target_bir_lowering=False)
    x_d = nc.dram_tensor("x", [BS, D], f32, kind="ExternalInput")
    w_d = nc.dram_tensor("weight", [M, D], f32, kind="ExternalInput")
    out_d = nc.dram_tensor("out", [BS, D], f32, kind="ExternalOutput")
    mw_d = nc.dram_tensor("mem_weight", [BS, M], f32, kind="ExternalOutput")
    whi_d = nc.dram_tensor("whi_scratch", [M, D], f16)  # internal scratch

    with TileContext(nc) as tc:
        with (
            tc.tile_pool(name="singles", bufs=1) as singles,
            tc.tile_pool(name="big8", bufs=1) as big8,
            tc.tile_pool(name="lbuf", bufs=1) as lbuf,
            tc.tile_pool(name="io4k", bufs=2) as io4k,
            tc.tile_pool(name="hi16", bufs=2) as hi16,
            tc.tile_pool(name="lo16", bufs=1) as lo16,
            tc.tile_pool(name="wtp", bufs=3) as wtp,
            tc.tile_pool(name="m16p", bufs=1) as m16p,
            tc.tile_pool(name="stage", bufs=2) as stage,
            tc.tile_pool(name="sc", bufs=2) as sc,
            tc.tile_pool(name="small", bufs=3) as small,
            tc.tile_pool(name="rl1p", bufs=4) as rl1p,
            tc.tile_pool(name="psA", bufs=2, space="PSUM") as psA,
            tc.tile_pool(name="psB", bufs=4, space="PSUM") as psB,
        ):
            ident16 = singles.tile([128, 128], f16)
            make_identity(nc, ident16)
            ident32 = singles.tile([128, 128], f32)
            make_identity(nc, ident32)
            junk = singles.tile([128, D // 4], f32)  # accum-only act sink

            # x^T hi/lo f16 [D_p, 2*kchunk, B_f] (slot reused for mm2 w16)
            xhT = big8.tile([128, 2 * NKC, BS], f16, tag="big")
            # logits -> (later) mem_weight, [128, bt, M]
            L = lbuf.tile([128, NBT, M], f32, tag="L")
            # mem_weight^T fp16 [M_p, mtile, B]
            mwT = singles.tile([128, NMT, BS], f16)

            # ---------------- Phase A: x load/norm/split/transpose --------
            def prep_x(bt):
                xt = io4k.tile([128, D], f32, tag="io")
                nc.sync.dma_start(out=xt, in_=x_d[bt * 128:(bt + 1) * 128, :])
                ss = small.tile([128, 4], f32, tag="ss")
                for q in range(4):
                    nc.scalar.activation(
                        out=junk, in_=xt[:, q * (D // 4):(q + 1) * (D // 4)],
                        func=AF.Square, accum_out=ss[:, q:q + 1])
                ssum = small.tile([128, 1], f32, tag="ssum")
                nc.vector.reduce_sum(ssum, ss, axis=AX.X)
                xn = small.tile([128, 1], f32, tag="xn")
                nc.scalar.sqrt(xn, ssum)
                xinv = small.tile([128, 1], f32, tag="xinv")
                nc.vector.reciprocal(xinv, xn)
                xh = io4k.tile([128, D], f32, tag="io")
                nc.vector.tensor_scalar_mul(xh, xt, xinv)  # normalized rows
                xhi = hi16.tile([128, D], f16, tag="hi")
                nc.scalar.copy(out=xhi, in_=xh)
                nc.vector.tensor_tensor(out=xh, in0=xh, in1=xhi,
                                        op=ALU.subtract)  # residual in place
                xlo = lo16.tile([128, D], f16, tag="lo")
                nc.vector.tensor_scalar_mul(xlo, xh, LO_SCALE)
                return xhi, xlo

            def transp_x(bt, xhi, xlo):
                for half, xsrc in ((0, xhi), (1, xlo)):
                    for kg in range(NKC // 4):
                        pt = psB.tile([128, 512], f16, tag="ptr16")
                        for c in range(4):
                            k = kg * 4 + c
                            nc.tensor.transpose(
                                pt[:, c * 128:(c + 1) * 128],
                                xsrc[:, k * 128:(k + 1) * 128], ident16)
                        nc.vector.tensor_copy(
                            out=xhT[:, half * NKC + kg * 4:
                                    half * NKC + (kg + 1) * 4,
                                    bt * 128:(bt + 1) * 128],
                            in_=pt.rearrange("p (c f) -> p c f", c=4))

            # ---------------- Phase B: mm1, software-pipelined ------------
            def prep_w(mt):
                mp = MT_SIZES[mt]
                moff = MT_OFFS[mt]
                wt = io4k.tile([128, D], f32, tag="io")
                nc.sync.dma_start(out=wt[:mp], in_=w_d[moff:moff + mp, :])
                wss = small.tile([128, 4], f32, tag="wss")
                for q in range(4):
                    nc.scalar.activation(
                        out=junk[:mp],
                        in_=wt[:mp, q * (D // 4):(q + 1) * (D // 4)],
                        func=AF.Square, accum_out=wss[:mp, q:q + 1])
                wsum = small.tile([128, 1], f32, tag="wsum")
                nc.vector.reduce_sum(wsum[:mp], wss[:mp], axis=AX.X)
                wn = small.tile([128, 1], f32, tag="wn")
                nc.scalar.sqrt(wn[:mp], wsum[:mp])
                winv = small.tile([128, 1], f32, tag="winv")
                nc.vector.reciprocal(winv[:mp], wn[:mp])
                whi = hi16.tile([128, D], f16, tag="hi")
                nc.scalar.copy(out=whi[:mp], in_=wt[:mp])
                # stash fp16 weight for mm2 rhs
                nc.sync.dma_start(out=whi_d[moff:moff + mp, :], in_=whi[:mp])
                nc.vector.tensor_tensor(out=wt[:mp], in0=wt[:mp],
                                        in1=whi[:mp], op=ALU.subtract)
                wlo = lo16.tile([128, D], f16, tag="lo")
                nc.vector.tensor_scalar_mul(wlo[:mp], wt[:mp], LO_SCALE)
                return whi, wlo, winv

            def mm1_tile(mt, whi, wlo, winv):
                mp = MT_SIZES[mt]
                moff = MT_OFFS[mt]
                pl = psA.tile([128, BS], f32, tag="pmm")
                plc = psA.tile([128, BS], f32, tag="pmmc")
                for kg in range(NKC // 4):
                    wTt = wtp.tile([128, 8, 128], f16, tag="wT")
                    for half, wsrc in ((0, whi), (1, wlo)):
                        pt = psB.tile([128, 512], f16, tag="ptr16")
                        for c in range(4):
                            k = kg * 4 + c
                            nc.tensor.transpose(
                                pt[:, c * 128:c * 128 + mp],
                                wsrc[:mp, k * 128:(k + 1) * 128],
                                ident16[:mp, :mp])
                        nc.scalar.copy(
                            out=wTt[:, half * 4:(half + 1) * 4, :mp],
                            in_=pt.rearrange("p (c f) -> p c f",
                                             c=4)[:, :, :mp])
                    for c in range(4):
                        k = kg * 4 + c
                        # T1: hi*hi
                        nc.tensor.matmul(
                            pl[:mp, :], lhsT=wTt[:, c, :mp],
                            rhs=xhT[:, k, :],
                            start=(k == 0), stop=(k == NKC - 1))
                        # cross terms (one psum group): hi_w*lo_x + lo_w*hi_x
                        nc.tensor.matmul(
                            plc[:mp, :], lhsT=wTt[:, c, :mp],
                            rhs=xhT[:, NKC + k, :],
                            start=(k == 0), stop=False)
                        nc.tensor.matmul(
                            plc[:mp, :], lhsT=wTt[:, 4 + c, :mp],
                            rhs=xhT[:, k, :],
                            start=False, stop=(k == NKC - 1))
                # combine hi*hi + cross/LO_SCALE, fold 1/||w||
                lt = stage.tile([128, BS], f32, tag="lt")
                nc.vector.tensor_scalar(
                    out=lt[:mp], in0=plc[:mp], scalar1=1.0 / LO_SCALE,
                    scalar2=winv[:mp], op0=ALU.mult, op1=ALU.mult)
                lt2 = stage.tile([128, BS], f32, tag="lt")
                nc.vector.scalar_tensor_tensor(
                    out=lt2[:mp], in0=pl[:mp], scalar=winv[:mp],
                    in1=lt[:mp], op0=ALU.mult, op1=ALU.add)
                # transpose logits^T -> logits in L
                pt2 = psB.tile([128, 512], f32, tag="ptr16")
                for btc in range(NBT):
                    nc.tensor.transpose(
                        pt2[:, btc * mp:(btc + 1) * mp],
                        lt2[:mp, btc * 128:(btc + 1) * 128],
                        ident32[:mp, :mp])
                nc.vector.tensor_copy(
                    out=L[:, :, moff:moff + mp],
                    in_=pt2[:, :NBT * mp].rearrange("p (c f) -> p c f",
                                                    c=NBT))

            # emit phases A+B with one-stage software pipelining
            xparts = {}
            for bt in range(NBT + 1):
                if bt < NBT:
                    xparts[bt] = prep_x(bt)
                if bt >= 1:
                    transp_x(bt - 1, *xparts.pop(bt - 1))
            wparts = {}
            for mt in range(NMT + 1):
                if mt < NMT:
                    wparts[mt] = prep_w(mt)
                if mt >= 1:
                    mm1_tile(mt - 1, *wparts.pop(mt - 1))

            # ---------------- Phase C/D: softmax/shrink/renorm ------------
            rL1s = []
            for bt in range(NBT):
                Lb = L[:, bt, :]
                rowmax = small.tile([128, 1], f32, tag="rmax")
                nc.vector.reduce_max(rowmax, Lb, axis=AX.X)
                negmax = small.tile([128, 1], f32, tag="nmax")
                nc.vector.tensor_scalar_mul(negmax, rowmax, -1.0)
                # pass 1: S = M + sum(exp - 1), fp32-exact
                sE4 = small.tile([128, NCH], f32, tag="sE4")
                for c in range(NCH):
                    cs = slice(c * CH, (c + 1) * CH)
                    e = sc.tile([128, CH], f32, tag="sa")
                    nc.scalar.activation(out=e, in_=Lb[:, cs], func=AF.Exp,
                                         bias=negmax, scale=1.0)
                    em1 = sc.tile([128, CH], f32, tag="sb")
                    nc.vector.tensor_scalar(
                        out=em1, in0=e, scalar1=-1.0, scalar2=0.0,
                        op0=ALU.add, op1=ALU.add,
                        accum_out=sE4[:, c:c + 1])
                Ssum = small.tile([128, 1], f32, tag="Ssum")
                nc.vector.reduce_sum(Ssum, sE4, axis=AX.X)
                S = small.tile([128, 1], f32, tag="S")
                nc.vector.tensor_scalar_add(S, Ssum, float(M))
                rS = small.tile([128, 1], f32, tag="rS")
                nc.vector.reciprocal(rS, S)
                # pass 2: shrinkage; raw weights overwrite logits in L
                m16 = m16p.tile([128, M], f16, tag="m16")
                L14 = small.tile([128, NCH], f32, tag="L14")
                for c in range(NCH):
                    cs = slice(c * CH, (c + 1) * CH)
                    e = sc.tile([128, CH], f32, tag="sa")
                    nc.scalar.activation(out=e, in_=Lb[:, cs], func=AF.Exp,
                                         bias=negmax, scale=1.0)
                    diff = sc.tile([128, CH], f32, tag="sb")
                    nc.vector.tensor_scalar(
                        out=diff, in0=e, scalar1=rS, scalar2=SHRINK_T,
                        op0=ALU.mult, op1=ALU.subtract)
                    den = sc.tile([128, CH], f32, tag="sd")
                    nc.scalar.activation(out=den, in_=diff, func=AF.Abs)
                    nc.vector.tensor_scalar_add(den, den, SHRINK_EPS)
                    nc.vector.reciprocal(den, den)
                    rel = sc.tile([128, CH], f32, tag="se")
                    nc.scalar.activation(out=rel, in_=diff, func=AF.Relu)
                    nc.vector.tensor_tensor(out=rel, in0=rel, in1=e,
                                            op=ALU.mult)
                    nc.vector.tensor_tensor(out=rel, in0=rel, in1=den,
                                            op=ALU.mult)
                    # raw = relu*e*rden*rS into L, with row-sum accumulation
                    nc.vector.tensor_scalar(
                        out=Lb[:, cs], in0=rel, scalar1=rS, scalar2=0.0,
                        op0=ALU.mult, op1=ALU.add,
                        accum_out=L14[:, c:c + 1])
                    # fp16 copy of RAW weights for mm2 (renorm folded into
                    # the mm2 output staging scale)
                    nc.scalar.copy(out=m16[:, cs], in_=Lb[:, cs])
                L1 = small.tile([128, 1], f32, tag="L1")
                nc.vector.reduce_sum(L1, L14, axis=AX.X)
                rL1 = rl1p.tile([128, 1], f32, tag="rL1")
                rL1s.append(rL1)
                nc.vector.reciprocal(rL1, L1)
                for c in range(NCH):
                    cs = slice(c * CH, (c + 1) * CH)
                    nc.vector.tensor_scalar_mul(Lb[:, cs], Lb[:, cs], rL1)
                nc.sync.dma_start(out=mw_d[bt * 128:(bt + 1) * 128, :], in_=Lb)
                # mem_weight^T tiles (fp16) for mm2 lhsT
                for tg in range(4):
                    pt = psB.tile([128, 512], f16, tag="ptr16")
                    for c in range(4):
                        mt = tg * 4 + c
                        mp = MT_SIZES[mt]
                        nc.tensor.transpose(
                            pt[:mp, c * 128:c * 128 + 128],
                            m16[:, MT_OFFS[mt]:MT_OFFS[mt] + mp],
                            ident16)
                    for c in range(4):
                        mt = tg * 4 + c
                        mp = MT_SIZES[mt]
                        eng = nc.vector.tensor_copy if c % 2 == 0 \
                            else nc.scalar.copy
                        eng(out=mwT[:mp, mt, bt * 128:(bt + 1) * 128],
                            in_=pt[:mp, c * 128:c * 128 + 128])

            # ---------------- Phase E: mm2 out = mem_weight @ w_hi --------
            NQ = 4
            DQ = D // NQ          # 1024 columns per quarter

            def load_q(q, w16_parts):
                for mt in range(NMT):
                    mp = MT_SIZES[mt]
                    moff = MT_OFFS[mt]
                    w16, mt0 = w16_parts[mt // 8] if len(w16_parts) == 2 \
                        else w16_parts[0]
                    nc.sync.dma_start(
                        out=w16[:mp, mt - mt0, :],
                        in_=whi_d[moff:moff + mp, q * DQ:(q + 1) * DQ])

            def mm2_q(q, w16_parts, bts=range(NBT)):
                for bt in bts:
                    for n in range(DQ // 512):
                        po = psA.tile([128, 512], f32,
                                      tag="pmm" if n == 0 else "pmmc")
                        for mt in range(NMT):
                            mp = MT_SIZES[mt]
                            w16, mt0 = w16_parts[mt // 8] \
                                if len(w16_parts) == 2 else w16_parts[0]
                            nc.tensor.matmul(
                                po, lhsT=mwT[:mp, mt, bt * 128:(bt + 1) * 128],
                                rhs=w16[:mp, mt - mt0, n * 512:(n + 1) * 512],
                                start=(mt == 0), stop=(mt == NMT - 1))
                        ost = stage.tile([128, 512], f32, tag="lt")
                        nc.scalar.mul(ost, po, rL1s[bt])
                        nc.sync.dma_start(
                            out=out_d[bt * 128:(bt + 1) * 128,
                                      q * DQ + n * 512:q * DQ + (n + 1) * 512],
                            in_=ost)

            # prefetch q0 + q1 as early as slots free (big8 after mm1,
            # io slots already free during the last mm1 tile)
            w16q0 = big8.tile([128, NMT, DQ], f16, tag="big")
            q0p = [(w16q0, 0)]
            load_q(0, q0p)
            w16q1a = io4k.tile([128, 8, DQ], f16, tag="io")
            w16q1b = io4k.tile([128, 8, DQ], f16, tag="io")
            q1p = [(w16q1a, 0), (w16q1b, 8)]
            load_q(1, q1p)
            for bt in range(NBT):
                mm2_q(0, q0p, bts=[bt])
                mm2_q(1, q1p, bts=[bt])
            w16q2 = lbuf.tile([128, NMT, DQ], f16, tag="L")
            q2p = [(w16q2, 0)]
            load_q(2, q2p)
            w16q3 = big8.tile([128, NMT, DQ], f16, tag="big")
            q3p = [(w16q3, 0)]
            load_q(3, q3p)
            mm2_q(2, q2p)
            mm2_q(3, q3p)
    nc.compile()
    return nc


def _get_nc():
    if "nc" not in _cache:
        _cache["nc"] = _build_nc()
    return _cache["nc"]


def kernel(x: np.ndarray, weight: np.ndarray):
    from concourse.bass_utils import run_bass_kernel_spmd

    x = np.ascontiguousarray(np.asarray(x, dtype=np.float32))
    weight = np.ascontiguousarray(np.asarray(weight, dtype=np.float32))
    nc = _get_nc()
    in_maps = [
        {"x": x[i * BS:(i + 1) * BS], "weight": weight} for i in range(NCORES)
    ]
    res = run_bass_kernel_spmd(nc, in_maps, core_ids=list(range(NCORES)))
    results = res.results
    output = np.concatenate([r["out"] for r in results], axis=0)
    mem_weight = np.concatenate([r["mem_weight"] for r in results], axis=0)
    return output, mem_weight


if __name__ == "__main__":
    xs = np.random.randn(B_FULL, D).astype(np.float32)
    ws = (np.random.randn(M, D) / np.sqrt(D)).astype(np.float32)
    o, mw = kernel(xs, ws)
    print(o.shape, mw.shape, o.dtype, mw.dtype)


# revision 25
# speedup vs baseline: 1.2106x; 1.2106x over previous
"""MemoryUnit kernel for Trainium2 (8 NeuronCores, data-parallel over batch).

Computes, for x [4096,4096] and weight [2000,4096] (fp32):
  logits = cos_sim(x, weight)           # [B, M]
  mem_weight = renorm(shrink(softmax(logits)))
  output = mem_weight @ weight          # [B, D]
Returns (output, mem_weight) like the reference.

Sharding: batch split 512 rows/core; weight replicated.
Per-core pipeline (v3):
  - x rows normalized, split into fp16 hi/lo pair (lo scaled by 2048),
    PE-transposed to x^T halves [D_p, B_f]
  - mm1 as three fp16 matmul passes (hi*hi + (hi*lo + lo*hi)/2048) with
    fp32 PSUM accumulation — numerically at the fp32 noise floor;
    weight tiles split+transposed on the fly, software-pipelined one
    tile ahead; weight-hi also written to a DRAM scratch for mm2;
    1/||w|| folded into the PSUM->SBUF combine
  - softmax with max-subtraction, two-pass exp (no full E buffer);
    row-sum computed as M + sum(exp-1) for fp32-exact accumulation;
    hard shrinkage + L1 renorm (all fp32)
  - mm2 (fp16 operands, fp32 accum): out = mem_weight @ weight_hi with
    mem_weight^T built via PE transposes and weight_hi streamed back
    from the DRAM scratch in four pipelined quarter-tiles
"""

import numpy as np

B_FULL, D, M = 4096, 4096, 2000
NCORES = 8
BS = B_FULL // NCORES          # 512 batch rows per core
NBT = BS // 128                # 4 batch partition-tiles
NKC = D // 128                 # 32 contraction chunks of 128
NMT = 16
MT_SIZES = [128] * 15 + [80]   # 2000 = 15*128 + 80
MT_OFFS = [sum(MT_SIZES[:i]) for i in range(NMT)]
SHRINK_T = 1.0 / M
SHRINK_EPS = 1e-12
CH = 500                       # softmax free-dim chunk (4 * 500 = 2000)
NCH = M // CH
LO_SCALE = 2048.0              # fp16 residual scaling (2^11)

_cache = {}


def _build_nc():
    import concourse.mybir as mybir
    from concourse import bacc
    from concourse.masks import make_identity
    from concourse.tile import TileContext

    f32 = mybir.dt.float32
    f16 = mybir.dt.float16
    AF = mybir.ActivationFunctionType
    ALU = mybir.AluOpType
    AX = mybir.AxisListType

    nc = bacc.Bacc("TRN2", target_bir_lowering=False)
    x_d = nc.dram_tensor("x", [BS, D], f32, kind="ExternalInput")
    w_d = nc.dram_tensor("weight", [M, D], f32, kind="ExternalInput")
    out_d = nc.dram_tensor("out", [BS, D], f32, kind="ExternalOutput")
    mw_d = nc.dram_tensor("mem_weight", [BS, M], f32, kind="ExternalOutput")
    whi_d = nc.dram_tensor("whi_scratch", [M, D], f16)  # internal scratch

    with TileContext(nc) as tc:
        with (
            tc.tile_pool(name="singles", bufs=1) as singles,
            tc.tile_pool(name="big8", bufs=1) as big8,
            tc.tile_pool(name="lbuf", bufs=1) as lbuf,
            tc.tile_pool(name="io4k", bufs=2) as io4k,
            tc.tile_pool(name="hi16", bufs=2) as hi16,
            tc.tile_pool(name="lo16", bufs=1) as lo16,
            tc.tile_pool(name="wtp", bufs=2) as wtp,
            tc.tile_pool(name="m16p", bufs=1) as m16p,
            tc.tile_pool(name="stage", bufs=2) as stage,
            tc.tile_pool(name="sc", bufs=2) as sc,
            tc.tile_pool(name="small", bufs=3) as small,
            tc.tile_pool(name="rl1p", bufs=4) as rl1p,
            tc.tile_pool(name="psA", bufs=2, space="PSUM") as psA,
            tc.tile_pool(name="psB", bufs=3, space="PSUM") as psB,
        ):
            ident16 = singles.tile([128, 128], f16)
            make_identity(nc, ident16)
            ident32 = singles.tile([128, 128], f32)
            make_identity(nc, ident32)
            junk = singles.tile([128, D // 4], f32)  # accum-only act sink

            # x^T hi/lo f16 [D_p, 2*kchunk, B_f] (slot reused for mm2 w16)
            xhT = big8.tile([128, 2 * NKC, BS], f16, tag="big")
            # logits -> (later) mem_weight, [128, bt, M]
            L = lbuf.tile([128, NBT, M], f32, tag="L")
            # mem_weight^T fp16 [M_p, mtile, B]
            mwT = singles.tile([128, NMT, BS], f16)

            # ---------------- Phase A: x load/norm/split/transpose --------
            def prep_x(bt):
                xt = io4k.tile([128, D], f32, tag="io")
                nc.sync.dma_start(out=xt, in_=x_d[bt * 128:(bt + 1) * 128, :])
                ss = small.tile([128, 4], f32, tag="ss")
                for q in range(4):
                    nc.scalar.activation(
                        out=junk, in_=xt[:, q * (D // 4):(q + 1) * (D // 4)],
                        func=AF.Square, accum_out=ss[:, q:q + 1])
                ssum = small.tile([128, 1], f32, tag="ssum")
                nc.vector.reduce_sum(ssum, ss, axis=AX.X)
                xn = small.tile([128, 1], f32, tag="xn")
                nc.scalar.sqrt(xn, ssum)
                xinv = small.tile([128, 1], f32, tag="xinv")
                nc.vector.reciprocal(xinv, xn)
                xh = io4k.tile([128, D], f32, tag="io")
                nc.vector.tensor_scalar_mul(xh, xt, xinv)  # normalized rows
                xhi = hi16.tile([128, D], f16, tag="hi")
                nc.scalar.copy(out=xhi, in_=xh)
                nc.vector.tensor_tensor(out=xh, in0=xh, in1=xhi,
                                        op=ALU.subtract)  # residual in place
                xlo = lo16.tile([128, D], f16, tag="lo")
                nc.vector.tensor_scalar_mul(xlo, xh, LO_SCALE)
                return xhi, xlo

            def transp_x(bt, xhi, xlo):
                for half, xsrc in ((0, xhi), (1, xlo)):
                    for kg in range(NKC // 4):
                        pt = psB.tile([128, 512], f16, tag="ptr16")
                        for c in range(4):
                            k = kg * 4 + c
                            nc.tensor.transpose(
                                pt[:, c * 128:(c + 1) * 128],
                                xsrc[:, k * 128:(k + 1) * 128], ident16)
                        nc.vector.tensor_copy(
                            out=xhT[:, half * NKC + kg * 4:
                                    half * NKC + (kg + 1) * 4,
                                    bt * 128:(bt + 1) * 128],
                            in_=pt.rearrange("p (c f) -> p c f", c=4))

            # ---------------- Phase B: mm1, software-pipelined ------------
            def prep_w(mt):
                mp = MT_SIZES[mt]
                moff = MT_OFFS[mt]
                wt = io4k.tile([128, D], f32, tag="io")
                nc.sync.dma_start(out=wt[:mp], in_=w_d[moff:moff + mp, :])
                wss = small.tile([128, 4], f32, tag="wss")
                for q in range(4):
                    nc.scalar.activation(
                        out=junk[:mp],
                        in_=wt[:mp, q * (D // 4):(q + 1) * (D // 4)],
                        func=AF.Square, accum_out=wss[:mp, q:q + 1])
                wsum = small.tile([128, 1], f32, tag="wsum")
                nc.vector.reduce_sum(wsum[:mp], wss[:mp], axis=AX.X)
                wn = small.tile([128, 1], f32, tag="wn")
                nc.scalar.sqrt(wn[:mp], wsum[:mp])
                winv = small.tile([128, 1], f32, tag="winv")
                nc.vector.reciprocal(winv[:mp], wn[:mp])
                whi = hi16.tile([128, D], f16, tag="hi")
                nc.scalar.copy(out=whi[:mp], in_=wt[:mp])
                # stash fp16 weight for mm2 rhs
                nc.sync.dma_start(out=whi_d[moff:moff + mp, :], in_=whi[:mp])
                nc.vector.tensor_tensor(out=wt[:mp], in0=wt[:mp],
                                        in1=whi[:mp], op=ALU.subtract)
                wlo = lo16.tile([128, D], f16, tag="lo")
                nc.vector.tensor_scalar_mul(wlo[:mp], wt[:mp], LO_SCALE)
                return whi, wlo, winv

            def mm1_tile(mt, whi, wlo, winv):
                mp = MT_SIZES[mt]
                moff = MT_OFFS[mt]
                pl = psA.tile([128, BS], f32, tag="pmm")
                plc = psA.tile([128, BS], f32, tag="pmmc")
                for kg in range(NKC // 4):
                    wTt = wtp.tile([128, 8, 128], f16, tag="wT")
                    for half, wsrc in ((0, whi), (1, wlo)):
                        pt = psB.tile([128, 512], f16, tag="ptr16")
                        for c in range(4):
                            k = kg * 4 + c
                            nc.tensor.transpose(
                                pt[:, c * 128:c * 128 + mp],
                                wsrc[:mp, k * 128:(k + 1) * 128],
                                ident16[:mp, :mp])
                        nc.scalar.copy(
                            out=wTt[:, half * 4:(half + 1) * 4, :mp],
                            in_=pt.rearrange("p (c f) -> p c f",
                                             c=4)[:, :, :mp])
                    for c in range(4):
                        k = kg * 4 + c
                        # T1: hi*hi
                        nc.tensor.matmul(
                            pl[:mp, :], lhsT=wTt[:, c, :mp],
                            rhs=xhT[:, k, :],
                            start=(k == 0), stop=(k == NKC - 1))
                        # cross terms (one psum group): hi_w*lo_x + lo_w*hi_x
                        nc.tensor.matmul(
                            plc[:mp, :], lhsT=wTt[:, c, :mp],
                            rhs=xhT[:, NKC + k, :],
                            start=(k == 0), stop=False)
                        nc.tensor.matmul(
                            plc[:mp, :], lhsT=wTt[:, 4 + c, :mp],
                            rhs=xhT[:, k, :],
                            start=False, stop=(k == NKC - 1))
                # combine hi*hi + cross/LO_SCALE, fold 1/||w||
                lt = stage.tile([128, BS], f32, tag="lt")
                nc.vector.tensor_scalar(
                    out=lt[:mp], in0=plc[:mp], scalar1=1.0 / LO_SCALE,
                    scalar2=winv[:mp], op0=ALU.mult, op1=ALU.mult)
                lt2 = stage.tile([128, BS], f32, tag="lt")
                nc.vector.scalar_tensor_tensor(
                    out=lt2[:mp], in0=pl[:mp], scalar=winv[:mp],
                    in1=lt[:mp], op0=ALU.mult, op1=ALU.add)
                # transpose logits^T -> logits in L
                pt2 = psB.tile([128, 512], f32, tag="ptr16")
                for btc in range(NBT):
                    nc.tensor.transpose(
                        pt2[:, btc * mp:(btc + 1) * mp],
                        lt2[:mp, btc * 128:(btc + 1) * 128],
                        ident32[:mp, :mp])
                nc.vector.tensor_copy(
                    out=L[:, :, moff:moff + mp],
                    in_=pt2[:, :NBT * mp].rearrange("p (c f) -> p c f",
                                                    c=NBT))

            # emit phases A+B with one-stage software pipelining
            xparts = {}
            for bt in range(NBT + 1):
                if bt < NBT:
                    xparts[bt] = prep_x(bt)
                if bt >= 1:
                    transp_x(bt - 1, *xparts.pop(bt - 1))
            wparts = {}
            for mt in range(NMT + 1):
                if mt < NMT:
                    wparts[mt] = prep_w(mt)
                if mt >= 1:
                    mm1_tile(mt - 1, *wparts.pop(mt - 1))

            # ---------------- Phase C/D: softmax/shrink/renorm ------------
            rL1s = []
            for bt in range(NBT):
                Lb = L[:, bt, :]
                rowmax = small.tile([128, 1], f32, tag="rmax")
                nc.vector.reduce_max(rowmax, Lb, axis=AX.X)
                negmax = small.tile([128, 1], f32, tag="nmax")
                nc.vector.tensor_scalar_mul(negmax, rowmax, -1.0)
                # pass 1: S = M + sum(exp - 1), fp32-exact
                sE4 = small.tile([128, NCH], f32, tag="sE4")
                for c in range(NCH):
                    cs = slice(c * CH, (c + 1) * CH)
                    e = sc.tile([128, CH], f32, tag="sa")
                    nc.scalar.activation(out=e, in_=Lb[:, cs], func=AF.Exp,
                                         bias=negmax, scale=1.0)
                    em1 = sc.tile([128, CH], f32, tag="sb")
                    nc.vector.tensor_scalar(
                        out=em1, in0=e, scalar1=-1.0, scalar2=0.0,
                        op0=ALU.add, op1=ALU.add,
                        accum_out=sE4[:, c:c + 1])
                Ssum = small.tile([128, 1], f32, tag="Ssum")
                nc.vector.reduce_sum(Ssum, sE4, axis=AX.X)
                S = small.tile([128, 1], f32, tag="S")
                nc.vector.tensor_scalar_add(S, Ssum, float(M))
                rS = small.tile([128, 1], f32, tag="rS")
                nc.vector.reciprocal(rS, S)
                # pass 2: shrinkage; raw weights overwrite logits in L
                m16 = m16p.tile([128, M], f16, tag="m16")
                L14 = small.tile([128, NCH], f32, tag="L14")
                for c in range(NCH):
                    cs = slice(c * CH, (c + 1) * CH)
                    e = sc.tile([128, CH], f32, tag="sa")
                    nc.scalar.activation(out=e, in_=Lb[:, cs], func=AF.Exp,
                                         bias=negmax, scale=1.0)
                    diff = sc.tile([128, CH], f32, tag="sb")
                    nc.vector.tensor_scalar(
                        out=diff, in0=e, scalar1=rS, scalar2=SHRINK_T,
                        op0=ALU.mult, op1=ALU.subtract)
                    den = sc.tile([128, CH], f32, tag="sd")
                    nc.scalar.activation(out=den, in_=diff, func=AF.Abs)
                    nc.vector.tensor_scalar_add(den, den, SHRINK_EPS)
                    nc.vector.reciprocal(den, den)
                    rel = sc.tile([128, CH], f32, tag="se")
                    nc.scalar.activation(out=rel, in_=diff, func=AF.Relu)
                    nc.vector.tensor_tensor(out=rel, in0=rel, in1=e,
                                            op=ALU.mult)
                    nc.vector.tensor_tensor(out=rel, in0=rel, in1=den,
                                            op=ALU.mult)
                    # raw = relu*e*rden*rS into L, with row-sum accumulation
                    nc.vector.tensor_scalar(
                        out=Lb[:, cs], in0=rel, scalar1=rS, scalar2=0.0,
                        op0=ALU.mult, op1=ALU.add,
                        accum_out=L14[:, c:c + 1])
                    # fp16 copy of RAW weights for mm2 (renorm folded into
                    # the mm2 output staging scale)
                    nc.scalar.copy(out=m16[:, cs], in_=Lb[:, cs])
                L1 = small.tile([128, 1], f32, tag="L1")
                nc.vector.reduce_sum(L1, L14, axis=AX.X)
                rL1 = rl1p.tile([128, 1], f32, tag="rL1")
                rL1s.append(rL1)
                nc.vector.reciprocal(rL1, L1)
                for c in range(NCH):
                    cs = slice(c * CH, (c + 1) * CH)
                    nc.vector.tensor_scalar_mul(Lb[:, cs], Lb[:, cs], rL1)
                nc.sync.dma_start(out=mw_d[bt * 128:(bt + 1) * 128, :], in_=Lb)
                # mem_weight^T tiles (fp16) for mm2 lhsT
                for tg in range(4):
                    pt = psB.tile([128, 512], f16, tag="ptr16")
                    for c in range(4):
                        mt = tg * 4 + c
                        mp = MT_SIZES[mt]
                        nc.tensor.transpose(
                            pt[:mp, c * 128:c * 128 + 128],
                            m16[:, MT_OFFS[mt]:MT_OFFS[mt] + mp],
                            ident16)
                    for c in range(4):
                        mt = tg * 4 + c
                        mp = MT_SIZES[mt]
                        eng = nc.vector.tensor_copy if c % 2 == 0 \
                            else nc.scalar.copy
                        eng(out=mwT[:mp, mt, bt * 128:(bt + 1) * 128],
                            in_=pt[:mp, c * 128:c * 128 + 128])

            # ---------------- Phase E: mm2 out = mem_weight @ w_hi --------
            NQ = 4
            DQ = D // NQ          # 1024 columns per quarter

            def load_q(q, w16_parts):
                for mt in range(NMT):
                    mp = MT_SIZES[mt]
                    moff = MT_OFFS[mt]
                    w16, mt0 = w16_parts[mt // 8] if len(w16_parts) == 2 \
                        else w16_parts[0]
                    nc.sync.dma_start(
                        out=w16[:mp, mt - mt0, :],
                        in_=whi_d[moff:moff + mp, q * DQ:(q + 1) * DQ])

            def mm2_q(q, w16_parts, bts=range(NBT)):
                for bt in bts:
                    for n in range(DQ // 512):
                        po = psA.tile([128, 512], f32,
                                      tag="pmm" if n == 0 else "pmmc")
                        for mt in range(NMT):
                            mp = MT_SIZES[mt]
                            w16, mt0 = w16_parts[mt // 8] \
                                if len(w16_parts) == 2 else w16_parts[0]
                            nc.tensor.matmul(
                                po, lhsT=mwT[:mp, mt, bt * 128:(bt + 1) * 128],
                                rhs=w16[:mp, mt - mt0, n * 512:(n + 1) * 512],
                                start=(mt == 0), stop=(mt == NMT - 1))
                        ost = stage.tile([128, 512], f32, tag="lt")
                        nc.scalar.mul(ost, po, rL1s[bt])
                        nc.sync.dma_start(
                            out=out_d[bt * 128:(bt + 1) * 128,
                                      q * DQ + n * 512:q * DQ + (n + 1) * 512],
                            in_=ost)

            # prefetch q0 + q1 as early as slots free (big8 after mm1,
            # io slots already free during the last mm1 tile)
            w16q0 = big8.tile([128, NMT, DQ], f16, tag="big")
            q0p = [(w16q0, 0)]
            load_q(0, q0p)
            w16q1a = io4k.tile([128, 8, DQ], f16, tag="io")
            w16q1b = io4k.tile([128, 8, DQ], f16, tag="io")
            q1p = [(w16q1a, 0), (w16q1b, 8)]
            load_q(1, q1p)
            for bt in range(NBT):
                mm2_q(0, q0p, bts=[bt])
                mm2_q(1, q1p, bts=[bt])
            w16q2 = lbuf.tile([128, NMT, DQ], f16, tag="L")
            q2p = [(w16q2, 0)]
            load_q(2, q2p)
            w16q3 = big8.tile([128, NMT, DQ], f16, tag="big")
            q3p = [(w16q3, 0)]
            load_q(3, q3p)
            mm2_q(2, q2p)
            mm2_q(3, q3p)
    nc.compile()
    return nc


def _get_nc():
    if "nc" not in _cache:
        _cache["nc"] = _build_nc()
    return _cache["nc"]


def kernel(x: np.ndarray, weight: np.ndarray):
    from concourse.bass_utils import run_bass_kernel_spmd

    x = np.ascontiguousarray(np.asarray(x, dtype=np.float32))
    weight = np.ascontiguousarray(np.asarray(weight, dtype=np.float32))
    nc = _get_nc()
    in_maps = [
        {"x": x[i * BS:(i + 1) * BS], "weight": weight} for i in range(NCORES)
    ]
    res = run_bass_kernel_spmd(nc, in_maps, core_ids=list(range(NCORES)))
    results = res.results
    output = np.concatenate([r["out"] for r in results], axis=0)
    mem_weight = np.concatenate([r["mem_weight"] for r in results], axis=0)
    return output, mem_weight


if __name__ == "__main__":
    xs = np.random.randn(B_FULL, D).astype(np.float32)
    ws = (np.random.randn(M, D) / np.sqrt(D)).astype(np.float32)
    o, mw = kernel(xs, ws)
    print(o.shape, mw.shape, o.dtype, mw.dtype)


# revision 26
# speedup vs baseline: 1.2209x; 1.0085x over previous
"""MemoryUnit kernel for Trainium2 (8 NeuronCores, data-parallel over batch).

Computes, for x [4096,4096] and weight [2000,4096] (fp32):
  logits = cos_sim(x, weight)           # [B, M]
  mem_weight = renorm(shrink(softmax(logits)))
  output = mem_weight @ weight          # [B, D]
Returns (output, mem_weight) like the reference.

Sharding: batch split 512 rows/core; weight replicated.
Per-core pipeline (v3):
  - x rows normalized, split into fp16 hi/lo pair (lo scaled by 2048),
    PE-transposed to x^T halves [D_p, B_f]
  - mm1 as three fp16 matmul passes (hi*hi + (hi*lo + lo*hi)/2048) with
    fp32 PSUM accumulation — numerically at the fp32 noise floor;
    weight tiles split+transposed on the fly, software-pipelined one
    tile ahead; weight-hi also written to a DRAM scratch for mm2;
    1/||w|| folded into the PSUM->SBUF combine
  - softmax with max-subtraction, two-pass exp (no full E buffer);
    row-sum computed as M + sum(exp-1) for fp32-exact accumulation;
    hard shrinkage + L1 renorm (all fp32)
  - mm2 (fp16 operands, fp32 accum): out = mem_weight @ weight_hi with
    mem_weight^T built via PE transposes and weight_hi streamed back
    from the DRAM scratch in four pipelined quarter-tiles
"""

import numpy as np

B_FULL, D, M = 4096, 4096, 2000
NCORES = 8
BS = B_FULL // NCORES          # 512 batch rows per core
NBT = BS // 128                # 4 batch partition-tiles
NKC = D // 128                 # 32 contraction chunks of 128
NMT = 16
MT_SIZES = [128] * 15 + [80]   # 2000 = 15*128 + 80
MT_OFFS = [sum(MT_SIZES[:i]) for i in range(NMT)]
SHRINK_T = 1.0 / M
SHRINK_EPS = 1e-12
CH = 500                       # softmax free-dim chunk (4 * 500 = 2000)
NCH = M // CH
LO_SCALE = 2048.0              # fp16 residual scaling (2^11)

_cache = {}


def _build_nc():
    import concourse.mybir as mybir
    from concourse import bacc
    from concourse.masks import make_identity
    from concourse.tile import TileContext

    f32 = mybir.dt.float32
    f16 = mybir.dt.float16
    AF = mybir.ActivationFunctionType
    ALU = mybir.AluOpType
    AX = mybir.AxisListType

    nc = bacc.Bacc("TRN2", target_bir_lowering=False)
    x_d = nc.dram_tensor("x", [BS, D], f32, kind="ExternalInput")
    w_d = nc.dram_tensor("weight", [M, D], f32, kind="ExternalInput")
    out_d = nc.dram_tensor("out", [BS, D], f32, kind="ExternalOutput")
    mw_d = nc.dram_tensor("mem_weight", [BS, M], f32, kind="ExternalOutput")
    whi_d = nc.dram_tensor("whi_scratch", [M, D], f16)  # internal scratch

    with TileContext(nc) as tc:
        with (
            tc.tile_pool(name="singles", bufs=1) as singles,
            tc.tile_pool(name="big8", bufs=1) as big8,
            tc.tile_pool(name="lbuf", bufs=1) as lbuf,
            tc.tile_pool(name="io4k", bufs=2) as io4k,
            tc.tile_pool(name="hi16", bufs=2) as hi16,
            tc.tile_pool(name="lo16", bufs=1) as lo16,
            tc.tile_pool(name="wtp", bufs=2) as wtp,
            tc.tile_pool(name="m16p", bufs=1) as m16p,
            tc.tile_pool(name="stage", bufs=3) as stage,
            tc.tile_pool(name="sc", bufs=2) as sc,
            tc.tile_pool(name="small", bufs=3) as small,
            tc.tile_pool(name="rl1p", bufs=4) as rl1p,
            tc.tile_pool(name="psA", bufs=2, space="PSUM") as psA,
            tc.tile_pool(name="psB", bufs=3, space="PSUM") as psB,
        ):
            ident16 = singles.tile([128, 128], f16)
            make_identity(nc, ident16)
            ident32 = singles.tile([128, 128], f32)
            make_identity(nc, ident32)
            junk = singles.tile([128, D // 4], f32)  # accum-only act sink

            # x^T hi/lo f16 [D_p, 2*kchunk, B_f] (slot reused for mm2 w16)
            xhT = big8.tile([128, 2 * NKC, BS], f16, tag="big")
            # logits -> (later) mem_weight, [128, bt, M]
            L = lbuf.tile([128, NBT, M], f32, tag="L")
            # mem_weight^T fp16 [M_p, mtile, B]
            mwT = singles.tile([128, NMT, BS], f16)

            # ---------------- Phase A: x load/norm/split/transpose --------
            def prep_x(bt):
                xt = io4k.tile([128, D], f32, tag="io")
                nc.sync.dma_start(out=xt, in_=x_d[bt * 128:(bt + 1) * 128, :])
                ss = small.tile([128, 4], f32, tag="ss")
                for q in range(4):
                    nc.scalar.activation(
                        out=junk, in_=xt[:, q * (D // 4):(q + 1) * (D // 4)],
                        func=AF.Square, accum_out=ss[:, q:q + 1])
                ssum = small.tile([128, 1], f32, tag="ssum")
                nc.vector.reduce_sum(ssum, ss, axis=AX.X)
                xn = small.tile([128, 1], f32, tag="xn")
                nc.scalar.sqrt(xn, ssum)
                xinv = small.tile([128, 1], f32, tag="xinv")
                nc.vector.reciprocal(xinv, xn)
                xh = io4k.tile([128, D], f32, tag="io")
                nc.vector.tensor_scalar_mul(xh, xt, xinv)  # normalized rows
                xhi = hi16.tile([128, D], f16, tag="hi")
                nc.scalar.copy(out=xhi, in_=xh)
                nc.vector.tensor_tensor(out=xh, in0=xh, in1=xhi,
                                        op=ALU.subtract)  # residual in place
                xlo = lo16.tile([128, D], f16, tag="lo")
                nc.vector.tensor_scalar_mul(xlo, xh, LO_SCALE)
                return xhi, xlo

            def transp_x(bt, xhi, xlo):
                for half, xsrc in ((0, xhi), (1, xlo)):
                    for kg in range(NKC // 4):
                        pt = psB.tile([128, 512], f16, tag="ptr16")
                        for c in range(4):
                            k = kg * 4 + c
                            nc.tensor.transpose(
                                pt[:, c * 128:(c + 1) * 128],
                                xsrc[:, k * 128:(k + 1) * 128], ident16)
                        nc.vector.tensor_copy(
                            out=xhT[:, half * NKC + kg * 4:
                                    half * NKC + (kg + 1) * 4,
                                    bt * 128:(bt + 1) * 128],
                            in_=pt.rearrange("p (c f) -> p c f", c=4))

            # ---------------- Phase B: mm1, software-pipelined ------------
            def prep_w(mt):
                mp = MT_SIZES[mt]
                moff = MT_OFFS[mt]
                wt = io4k.tile([128, D], f32, tag="io")
                nc.sync.dma_start(out=wt[:mp], in_=w_d[moff:moff + mp, :])
                wss = small.tile([128, 4], f32, tag="wss")
                for q in range(4):
                    nc.scalar.activation(
                        out=junk[:mp],
                        in_=wt[:mp, q * (D // 4):(q + 1) * (D // 4)],
                        func=AF.Square, accum_out=wss[:mp, q:q + 1])
                wsum = small.tile([128, 1], f32, tag="wsum")
                nc.vector.reduce_sum(wsum[:mp], wss[:mp], axis=AX.X)
                wn = small.tile([128, 1], f32, tag="wn")
                nc.scalar.sqrt(wn[:mp], wsum[:mp])
                winv = small.tile([128, 1], f32, tag="winv")
                nc.vector.reciprocal(winv[:mp], wn[:mp])
                whi = hi16.tile([128, D], f16, tag="hi")
                nc.scalar.copy(out=whi[:mp], in_=wt[:mp])
                # stash fp16 weight for mm2 rhs
                nc.sync.dma_start(out=whi_d[moff:moff + mp, :], in_=whi[:mp])
                nc.vector.tensor_tensor(out=wt[:mp], in0=wt[:mp],
                                        in1=whi[:mp], op=ALU.subtract)
                wlo = lo16.tile([128, D], f16, tag="lo")
                nc.vector.tensor_scalar_mul(wlo[:mp], wt[:mp], LO_SCALE)
                return whi, wlo, winv

            def mm1_tile(mt, whi, wlo, winv):
                mp = MT_SIZES[mt]
                moff = MT_OFFS[mt]
                pl = psA.tile([128, BS], f32, tag="pmm")
                plc = psA.tile([128, BS], f32, tag="pmmc")
                for kg in range(NKC // 4):
                    wTt = wtp.tile([128, 8, 128], f16, tag="wT")
                    for half, wsrc in ((0, whi), (1, wlo)):
                        pt = psB.tile([128, 512], f16, tag="ptr16")
                        for c in range(4):
                            k = kg * 4 + c
                            nc.tensor.transpose(
                                pt[:, c * 128:c * 128 + mp],
                                wsrc[:mp, k * 128:(k + 1) * 128],
                                ident16[:mp, :mp])
                        nc.scalar.copy(
                            out=wTt[:, half * 4:(half + 1) * 4, :mp],
                            in_=pt.rearrange("p (c f) -> p c f",
                                             c=4)[:, :, :mp])
                    for c in range(4):
                        k = kg * 4 + c
                        # T1: hi*hi
                        nc.tensor.matmul(
                            pl[:mp, :], lhsT=wTt[:, c, :mp],
                            rhs=xhT[:, k, :],
                            start=(k == 0), stop=(k == NKC - 1))
                        # cross terms (one psum group): hi_w*lo_x + lo_w*hi_x
                        nc.tensor.matmul(
                            plc[:mp, :], lhsT=wTt[:, c, :mp],
                            rhs=xhT[:, NKC + k, :],
                            start=(k == 0), stop=False)
                        nc.tensor.matmul(
                            plc[:mp, :], lhsT=wTt[:, 4 + c, :mp],
                            rhs=xhT[:, k, :],
                            start=False, stop=(k == NKC - 1))
                # combine hi*hi + cross/LO_SCALE, fold 1/||w||
                lt = stage.tile([128, BS], f32, tag="lt")
                nc.vector.tensor_scalar(
                    out=lt[:mp], in0=plc[:mp], scalar1=1.0 / LO_SCALE,
                    scalar2=winv[:mp], op0=ALU.mult, op1=ALU.mult)
                lt2 = stage.tile([128, BS], f32, tag="lt")
                nc.vector.scalar_tensor_tensor(
                    out=lt2[:mp], in0=pl[:mp], scalar=winv[:mp],
                    in1=lt[:mp], op0=ALU.mult, op1=ALU.add)
                # transpose logits^T -> logits in L
                pt2 = psB.tile([128, 512], f32, tag="ptr16")
                for btc in range(NBT):
                    nc.tensor.transpose(
                        pt2[:, btc * mp:(btc + 1) * mp],
                        lt2[:mp, btc * 128:(btc + 1) * 128],
                        ident32[:mp, :mp])
                nc.vector.tensor_copy(
                    out=L[:, :, moff:moff + mp],
                    in_=pt2[:, :NBT * mp].rearrange("p (c f) -> p c f",
                                                    c=NBT))

            # emit phases A+B with one-stage software pipelining
            xparts = {}
            for bt in range(NBT + 1):
                if bt < NBT:
                    xparts[bt] = prep_x(bt)
                if bt >= 1:
                    transp_x(bt - 1, *xparts.pop(bt - 1))
            wparts = {}
            for mt in range(NMT + 1):
                if mt < NMT:
                    wparts[mt] = prep_w(mt)
                if mt >= 1:
                    mm1_tile(mt - 1, *wparts.pop(mt - 1))

            # ---------------- Phase C/D: softmax/shrink/renorm ------------
            rL1s = []
            for bt in range(NBT):
                Lb = L[:, bt, :]
                rowmax = small.tile([128, 1], f32, tag="rmax")
                nc.vector.reduce_max(rowmax, Lb, axis=AX.X)
                negmax = small.tile([128, 1], f32, tag="nmax")
                nc.vector.tensor_scalar_mul(negmax, rowmax, -1.0)
                # pass 1: S = M + sum(exp - 1), fp32-exact
                sE4 = small.tile([128, NCH], f32, tag="sE4")
                for c in range(NCH):
                    cs = slice(c * CH, (c + 1) * CH)
                    e = sc.tile([128, CH], f32, tag="sa")
                    nc.scalar.activation(out=e, in_=Lb[:, cs], func=AF.Exp,
                                         bias=negmax, scale=1.0)
                    em1 = sc.tile([128, CH], f32, tag="sb")
                    nc.vector.tensor_scalar(
                        out=em1, in0=e, scalar1=-1.0, scalar2=0.0,
                        op0=ALU.add, op1=ALU.add,
                        accum_out=sE4[:, c:c + 1])
                Ssum = small.tile([128, 1], f32, tag="Ssum")
                nc.vector.reduce_sum(Ssum, sE4, axis=AX.X)
                S = small.tile([128, 1], f32, tag="S")
                nc.vector.tensor_scalar_add(S, Ssum, float(M))
                rS = small.tile([128, 1], f32, tag="rS")
                nc.vector.reciprocal(rS, S)
                # pass 2: shrinkage; raw weights overwrite logits in L
                m16 = m16p.tile([128, M], f16, tag="m16")
                L14 = small.tile([128, NCH], f32, tag="L14")
                for c in range(NCH):
                    cs = slice(c * CH, (c + 1) * CH)
                    e = sc.tile([128, CH], f32, tag="sa")
                    nc.scalar.activation(out=e, in_=Lb[:, cs], func=AF.Exp,
                                         bias=negmax, scale=1.0)
                    diff = sc.tile([128, CH], f32, tag="sb")
                    nc.vector.tensor_scalar(
                        out=diff, in0=e, scalar1=rS, scalar2=SHRINK_T,
                        op0=ALU.mult, op1=ALU.subtract)
                    den = sc.tile([128, CH], f32, tag="sd")
                    nc.scalar.activation(out=den, in_=diff, func=AF.Abs)
                    nc.vector.tensor_scalar_add(den, den, SHRINK_EPS)
                    nc.vector.reciprocal(den, den)
                    rel = sc.tile([128, CH], f32, tag="se")
                    nc.scalar.activation(out=rel, in_=diff, func=AF.Relu)
                    nc.vector.tensor_tensor(out=rel, in0=rel, in1=e,
                                            op=ALU.mult)
                    nc.vector.tensor_tensor(out=rel, in0=rel, in1=den,
                                            op=ALU.mult)
                    # raw = relu*e*rden*rS into L, with row-sum accumulation
                    nc.vector.tensor_scalar(
                        out=Lb[:, cs], in0=rel, scalar1=rS, scalar2=0.0,
                        op0=ALU.mult, op1=ALU.add,
                        accum_out=L14[:, c:c + 1])
                    # fp16 copy of RAW weights for mm2 (renorm folded into
                    # the mm2 output staging scale)
                    nc.scalar.copy(out=m16[:, cs], in_=Lb[:, cs])
                L1 = small.tile([128, 1], f32, tag="L1")
                nc.vector.reduce_sum(L1, L14, axis=AX.X)
                rL1 = rl1p.tile([128, 1], f32, tag="rL1")
                rL1s.append(rL1)
                nc.vector.reciprocal(rL1, L1)
                for c in range(NCH):
                    cs = slice(c * CH, (c + 1) * CH)
                    nc.vector.tensor_scalar_mul(Lb[:, cs], Lb[:, cs], rL1)
                nc.sync.dma_start(out=mw_d[bt * 128:(bt + 1) * 128, :], in_=Lb)
                # mem_weight^T tiles (fp16) for mm2 lhsT
                for tg in range(4):
                    pt = psB.tile([128, 512], f16, tag="ptr16")
                    for c in range(4):
                        mt = tg * 4 + c
                        mp = MT_SIZES[mt]
                        nc.tensor.transpose(
                            pt[:mp, c * 128:c * 128 + 128],
                            m16[:, MT_OFFS[mt]:MT_OFFS[mt] + mp],
                            ident16)
                    for c in range(4):
                        mt = tg * 4 + c
                        mp = MT_SIZES[mt]
                        eng = nc.vector.tensor_copy if c % 2 == 0 \
                            else nc.scalar.copy
                        eng(out=mwT[:mp, mt, bt * 128:(bt + 1) * 128],
                            in_=pt[:mp, c * 128:c * 128 + 128])

            # ---------------- Phase E: mm2 out = mem_weight @ w_hi --------
            NQ = 4
            DQ = D // NQ          # 1024 columns per quarter

            def load_q(q, w16_parts):
                for mt in range(NMT):
                    mp = MT_SIZES[mt]
                    moff = MT_OFFS[mt]
                    w16, mt0 = w16_parts[mt // 8] if len(w16_parts) == 2 \
                        else w16_parts[0]
                    nc.sync.dma_start(
                        out=w16[:mp, mt - mt0, :],
                        in_=whi_d[moff:moff + mp, q * DQ:(q + 1) * DQ])

            def mm2_q(q, w16_parts, bts=range(NBT)):
                for bt in bts:
                    for n in range(DQ // 512):
                        po = psA.tile([128, 512], f32,
                                      tag="pmm" if n == 0 else "pmmc")
                        for mt in range(NMT):
                            mp = MT_SIZES[mt]
                            w16, mt0 = w16_parts[mt // 8] \
                                if len(w16_parts) == 2 else w16_parts[0]
                            nc.tensor.matmul(
                                po, lhsT=mwT[:mp, mt, bt * 128:(bt + 1) * 128],
                                rhs=w16[:mp, mt - mt0, n * 512:(n + 1) * 512],
                                start=(mt == 0), stop=(mt == NMT - 1))
                        ost = stage.tile([128, 512], f32, tag="lt")
                        nc.scalar.mul(ost, po, rL1s[bt])
                        nc.sync.dma_start(
                            out=out_d[bt * 128:(bt + 1) * 128,
                                      q * DQ + n * 512:q * DQ + (n + 1) * 512],
                            in_=ost)

            # prefetch q0 + q1 as early as slots free (big8 after mm1,
            # io slots already free during the last mm1 tile)
            w16q0 = big8.tile([128, NMT, DQ], f16, tag="big")
            q0p = [(w16q0, 0)]
            load_q(0, q0p)
            w16q1a = io4k.tile([128, 8, DQ], f16, tag="io")
            w16q1b = io4k.tile([128, 8, DQ], f16, tag="io")
            q1p = [(w16q1a, 0), (w16q1b, 8)]
            load_q(1, q1p)
            for bt in range(NBT):
                mm2_q(0, q0p, bts=[bt])
                mm2_q(1, q1p, bts=[bt])
            w16q2 = lbuf.tile([128, NMT, DQ], f16, tag="L")
            q2p = [(w16q2, 0)]
            load_q(2, q2p)
            w16q3 = big8.tile([128, NMT, DQ], f16, tag="big")
            q3p = [(w16q3, 0)]
            load_q(3, q3p)
            mm2_q(2, q2p)
            mm2_q(3, q3p)
    nc.compile()
    return nc


def _get_nc():
    if "nc" not in _cache:
        _cache["nc"] = _build_nc()
    return _cache["nc"]


def kernel(x: np.ndarray, weight: np.ndarray):
    from concourse.bass_utils import run_bass_kernel_spmd

    x = np.ascontiguousarray(np.asarray(x, dtype=np.float32))
    weight = np.ascontiguousarray(np.asarray(weight, dtype=np.float32))
    nc = _get_nc()
    in_maps = [
        {"x": x[i * BS:(i + 1) * BS], "weight": weight} for i in range(NCORES)
    ]
    res = run_bass_kernel_spmd(nc, in_maps, core_ids=list(range(NCORES)))
    results = res.results
    output = np.concatenate([r["out"] for r in results], axis=0)
    mem_weight = np.concatenate([r["mem_weight"] for r in results], axis=0)
    return output, mem_weight


if __name__ == "__main__":
    xs = np.random.randn(B_FULL, D).astype(np.float32)
    ws = (np.random.randn(M, D) / np.sqrt(D)).astype(np.float32)
    o, mw = kernel(xs, ws)
    print(o.shape, mw.shape, o.dtype, mw.dtype)


# revision 31
# speedup vs baseline: 1.2213x; 1.0003x over previous
"""MemoryUnit kernel for Trainium2 (8 NeuronCores, data-parallel over batch).

Computes, for x [4096,4096] and weight [2000,4096] (fp32):
  logits = cos_sim(x, weight)           # [B, M]
  mem_weight = renorm(shrink(softmax(logits)))
  output = mem_weight @ weight          # [B, D]
Returns (output, mem_weight) like the reference.

Sharding: batch split 512 rows/core; weight replicated.
Per-core pipeline (v3):
  - x rows normalized, split into fp16 hi/lo pair (lo scaled by 2048),
    PE-transposed to x^T halves [D_p, B_f]
  - mm1 as three fp16 matmul passes (hi*hi + (hi*lo + lo*hi)/2048) with
    fp32 PSUM accumulation — numerically at the fp32 noise floor;
    weight tiles split+transposed on the fly, software-pipelined one
    tile ahead; weight-hi also written to a DRAM scratch for mm2;
    1/||w|| folded into the PSUM->SBUF combine
  - softmax with max-subtraction, two-pass exp (no full E buffer);
    row-sum computed as M + sum(exp-1) for fp32-exact accumulation;
    hard shrinkage + L1 renorm (all fp32)
  - mm2 (fp16 operands, fp32 accum): out = mem_weight @ weight_hi with
    mem_weight^T built via PE transposes and weight_hi streamed back
    from the DRAM scratch in four pipelined quarter-tiles
"""

import numpy as np

B_FULL, D, M = 4096, 4096, 2000
NCORES = 8
BS = B_FULL // NCORES          # 512 batch rows per core
NBT = BS // 128                # 4 batch partition-tiles
NKC = D // 128                 # 32 contraction chunks of 128
NMT = 16
MT_SIZES = [128] * 15 + [80]   # 2000 = 15*128 + 80
MT_OFFS = [sum(MT_SIZES[:i]) for i in range(NMT)]
SHRINK_T = 1.0 / M
SHRINK_EPS = 1e-12
CH = 500                       # softmax free-dim chunk (4 * 500 = 2000)
NCH = M // CH
LO_SCALE = 2048.0              # fp16 residual scaling (2^11)

_cache = {}


def _build_nc():
    import concourse.mybir as mybir
    from concourse import bacc
    from concourse.masks import make_identity
    from concourse.tile import TileContext

    f32 = mybir.dt.float32
    f16 = mybir.dt.float16
    AF = mybir.ActivationFunctionType
    ALU = mybir.AluOpType
    AX = mybir.AxisListType

    nc = bacc.Bacc("TRN2", target_bir_lowering=False)
    x_d = nc.dram_tensor("x", [BS, D], f32, kind="ExternalInput")
    w_d = nc.dram_tensor("weight", [M, D], f32, kind="ExternalInput")
    out_d = nc.dram_tensor("out", [BS, D], f32, kind="ExternalOutput")
    mw_d = nc.dram_tensor("mem_weight", [BS, M], f32, kind="ExternalOutput")
    whi_d = nc.dram_tensor("whi_scratch", [M, D], f16)  # internal scratch

    with TileContext(nc) as tc:
        with (
            tc.tile_pool(name="singles", bufs=1) as singles,
            tc.tile_pool(name="big8", bufs=1) as big8,
            tc.tile_pool(name="lbuf", bufs=1) as lbuf,
            tc.tile_pool(name="io4k", bufs=2) as io4k,
            tc.tile_pool(name="hi16", bufs=2) as hi16,
            tc.tile_pool(name="lo16", bufs=1) as lo16,
            tc.tile_pool(name="wtp", bufs=2) as wtp,
            tc.tile_pool(name="m16p", bufs=1) as m16p,
            tc.tile_pool(name="stage", bufs=3) as stage,
            tc.tile_pool(name="sc", bufs=2) as sc,
            tc.tile_pool(name="small", bufs=3) as small,
            tc.tile_pool(name="rl1p", bufs=4) as rl1p,
            tc.tile_pool(name="psA", bufs=2, space="PSUM") as psA,
            tc.tile_pool(name="psB", bufs=3, space="PSUM") as psB,
        ):
            ident16 = singles.tile([128, 128], f16)
            make_identity(nc, ident16)
            ident32 = singles.tile([128, 128], f32)
            make_identity(nc, ident32)
            junk = singles.tile([128, D // 4], f32)  # accum-only act sink

            # x^T hi/lo f16 [D_p, 2*kchunk, B_f] (slot reused for mm2 w16)
            xhT = big8.tile([128, 2 * NKC, BS], f16, tag="big")
            # logits -> (later) mem_weight, [128, bt, M]
            L = lbuf.tile([128, NBT, M], f32, tag="L")
            # mem_weight^T fp16 [M_p, mtile, B]
            mwT = singles.tile([128, NMT, BS], f16)

            # ---------------- Phase A: x load/norm/split/transpose --------
            def prep_x(bt):
                xt = io4k.tile([128, D], f32, tag="io")
                nc.sync.dma_start(out=xt, in_=x_d[bt * 128:(bt + 1) * 128, :])
                ss = small.tile([128, 4], f32, tag="ss")
                for q in range(4):
                    nc.scalar.activation(
                        out=junk, in_=xt[:, q * (D // 4):(q + 1) * (D // 4)],
                        func=AF.Square, accum_out=ss[:, q:q + 1])
                ssum = small.tile([128, 1], f32, tag="ssum")
                nc.vector.reduce_sum(ssum, ss, axis=AX.X)
                xn = small.tile([128, 1], f32, tag="xn")
                nc.scalar.sqrt(xn, ssum)
                xinv = small.tile([128, 1], f32, tag="xinv")
                nc.vector.reciprocal(xinv, xn)
                xh = io4k.tile([128, D], f32, tag="io")
                nc.vector.tensor_scalar_mul(xh, xt, xinv)  # normalized rows
                xhi = hi16.tile([128, D], f16, tag="hi")
                nc.scalar.copy(out=xhi, in_=xh)
                nc.vector.tensor_tensor(out=xh, in0=xh, in1=xhi,
                                        op=ALU.subtract)  # residual in place
                xlo = lo16.tile([128, D], f16, tag="lo")
                nc.vector.tensor_scalar_mul(xlo, xh, LO_SCALE)
                return xhi, xlo

            def transp_x(bt, xhi, xlo):
                for half, xsrc in ((0, xhi), (1, xlo)):
                    for kg in range(NKC // 4):
                        pt = psB.tile([128, 512], f16, tag="ptr16")
                        for c in range(4):
                            k = kg * 4 + c
                            nc.tensor.transpose(
                                pt[:, c * 128:(c + 1) * 128],
                                xsrc[:, k * 128:(k + 1) * 128], ident16)
                        nc.vector.tensor_copy(
                            out=xhT[:, half * NKC + kg * 4:
                                    half * NKC + (kg + 1) * 4,
                                    bt * 128:(bt + 1) * 128],
                            in_=pt.rearrange("p (c f) -> p c f", c=4))

            # ---------------- Phase B: mm1, software-pipelined ------------
            def prep_w(mt):
                mp = MT_SIZES[mt]
                moff = MT_OFFS[mt]
                wt = io4k.tile([128, D], f32, tag="io")
                nc.sync.dma_start(out=wt[:mp], in_=w_d[moff:moff + mp, :])
                wss = small.tile([128, 4], f32, tag="wss")
                for q in range(4):
                    nc.scalar.activation(
                        out=junk[:mp],
                        in_=wt[:mp, q * (D // 4):(q + 1) * (D // 4)],
                        func=AF.Square, accum_out=wss[:mp, q:q + 1])
                wsum = small.tile([128, 1], f32, tag="wsum")
                nc.vector.reduce_sum(wsum[:mp], wss[:mp], axis=AX.X)
                wn = small.tile([128, 1], f32, tag="wn")
                nc.scalar.sqrt(wn[:mp], wsum[:mp])
                winv = small.tile([128, 1], f32, tag="winv")
                nc.vector.reciprocal(winv[:mp], wn[:mp])
                whi = hi16.tile([128, D], f16, tag="hi")
                nc.scalar.copy(out=whi[:mp], in_=wt[:mp])
                # stash fp16 weight for mm2 rhs
                nc.sync.dma_start(out=whi_d[moff:moff + mp, :], in_=whi[:mp])
                nc.vector.tensor_tensor(out=wt[:mp], in0=wt[:mp],
                                        in1=whi[:mp], op=ALU.subtract)
                wlo = lo16.tile([128, D], f16, tag="lo")
                nc.vector.tensor_scalar_mul(wlo[:mp], wt[:mp], LO_SCALE)
                return whi, wlo, winv

            def mm1_tile(mt, whi, wlo, winv):
                mp = MT_SIZES[mt]
                moff = MT_OFFS[mt]
                pl = psA.tile([128, BS], f32, tag="pmm")
                plc = psA.tile([128, BS], f32, tag="pmmc")
                for kg in range(NKC // 4):
                    wTt = wtp.tile([128, 8, 128], f16, tag="wT")
                    for half, wsrc in ((0, whi), (1, wlo)):
                        pt = psB.tile([128, 512], f16, tag="ptr16")
                        for c in range(4):
                            k = kg * 4 + c
                            nc.tensor.transpose(
                                pt[:, c * 128:c * 128 + mp],
                                wsrc[:mp, k * 128:(k + 1) * 128],
                                ident16[:mp, :mp])
                        nc.scalar.copy(
                            out=wTt[:, half * 4:(half + 1) * 4, :mp],
                            in_=pt.rearrange("p (c f) -> p c f",
                                             c=4)[:, :, :mp])
                    for c in range(4):
                        k = kg * 4 + c
                        # T1: hi*hi
                        nc.tensor.matmul(
                            pl[:mp, :], lhsT=wTt[:, c, :mp],
                            rhs=xhT[:, k, :],
                            start=(k == 0), stop=(k == NKC - 1))
                        # cross terms (one psum group): hi_w*lo_x + lo_w*hi_x
                        nc.tensor.matmul(
                            plc[:mp, :], lhsT=wTt[:, c, :mp],
                            rhs=xhT[:, NKC + k, :],
                            start=(k == 0), stop=False)
                        nc.tensor.matmul(
                            plc[:mp, :], lhsT=wTt[:, 4 + c, :mp],
                            rhs=xhT[:, k, :],
                            start=False, stop=(k == NKC - 1))
                # combine hi*hi + cross/LO_SCALE, fold 1/||w||
                lt = stage.tile([128, BS], f32, tag="lt")
                nc.vector.tensor_scalar(
                    out=lt[:mp], in0=plc[:mp], scalar1=1.0 / LO_SCALE,
                    scalar2=winv[:mp], op0=ALU.mult, op1=ALU.mult)
                lt2 = stage.tile([128, BS], f32, tag="lt")
                nc.vector.scalar_tensor_tensor(
                    out=lt2[:mp], in0=pl[:mp], scalar=winv[:mp],
                    in1=lt[:mp], op0=ALU.mult, op1=ALU.add)
                # transpose logits^T -> logits in L
                pt2 = psB.tile([128, 512], f32, tag="ptr16")
                for btc in range(NBT):
                    nc.tensor.transpose(
                        pt2[:, btc * mp:(btc + 1) * mp],
                        lt2[:mp, btc * 128:(btc + 1) * 128],
                        ident32[:mp, :mp])
                nc.vector.tensor_copy(
                    out=L[:, :, moff:moff + mp],
                    in_=pt2[:, :NBT * mp].rearrange("p (c f) -> p c f",
                                                    c=NBT))

            # emit phases A+B with one-stage software pipelining; the
            # first two weight preps interleave into phase A's transposes
            xparts = {}
            wparts = {}
            xparts[0] = prep_x(0)
            xparts[1] = prep_x(1)
            transp_x(0, *xparts.pop(0))
            xparts[2] = prep_x(2)
            transp_x(1, *xparts.pop(1))
            xparts[3] = prep_x(3)
            transp_x(2, *xparts.pop(2))
            wparts[0] = prep_w(0)
            transp_x(3, *xparts.pop(3))
            wparts[1] = prep_w(1)
            for mt in range(NMT):
                mm1_tile(mt, *wparts.pop(mt))
                if mt + 2 < NMT:
                    wparts[mt + 2] = prep_w(mt + 2)

            # ---------------- Phase C/D: softmax/shrink/renorm ------------
            rL1s = []
            for bt in range(NBT):
                Lb = L[:, bt, :]
                rowmax = small.tile([128, 1], f32, tag="rmax")
                nc.vector.reduce_max(rowmax, Lb, axis=AX.X)
                negmax = small.tile([128, 1], f32, tag="nmax")
                nc.vector.tensor_scalar_mul(negmax, rowmax, -1.0)
                # pass 1: S = M + sum(exp - 1), fp32-exact
                sE4 = small.tile([128, NCH], f32, tag="sE4")
                for c in range(NCH):
                    cs = slice(c * CH, (c + 1) * CH)
                    e = sc.tile([128, CH], f32, tag="sa")
                    nc.scalar.activation(out=e, in_=Lb[:, cs], func=AF.Exp,
                                         bias=negmax, scale=1.0)
                    em1 = sc.tile([128, CH], f32, tag="sb")
                    nc.vector.tensor_scalar(
                        out=em1, in0=e, scalar1=-1.0, scalar2=0.0,
                        op0=ALU.add, op1=ALU.add,
                        accum_out=sE4[:, c:c + 1])
                Ssum = small.tile([128, 1], f32, tag="Ssum")
                nc.vector.reduce_sum(Ssum, sE4, axis=AX.X)
                S = small.tile([128, 1], f32, tag="S")
                nc.vector.tensor_scalar_add(S, Ssum, float(M))
                rS = small.tile([128, 1], f32, tag="rS")
                nc.vector.reciprocal(rS, S)
                # pass 2: shrinkage; raw weights overwrite logits in L
                m16 = m16p.tile([128, M], f16, tag="m16")
                L14 = small.tile([128, NCH], f32, tag="L14")
                for c in range(NCH):
                    cs = slice(c * CH, (c + 1) * CH)
                    e = sc.tile([128, CH], f32, tag="sa")
                    nc.scalar.activation(out=e, in_=Lb[:, cs], func=AF.Exp,
                                         bias=negmax, scale=1.0)
                    diff = sc.tile([128, CH], f32, tag="sb")
                    nc.vector.tensor_scalar(
                        out=diff, in0=e, scalar1=rS, scalar2=SHRINK_T,
                        op0=ALU.mult, op1=ALU.subtract)
                    den = sc.tile([128, CH], f32, tag="sd")
                    nc.scalar.activation(out=den, in_=diff, func=AF.Abs)
                    nc.vector.tensor_scalar_add(den, den, SHRINK_EPS)
                    nc.vector.reciprocal(den, den)
                    rel = sc.tile([128, CH], f32, tag="se")
                    nc.scalar.activation(out=rel, in_=diff, func=AF.Relu)
                    nc.vector.tensor_tensor(out=rel, in0=rel, in1=e,
                                            op=ALU.mult)
                    nc.vector.tensor_tensor(out=rel, in0=rel, in1=den,
                                            op=ALU.mult)
                    # raw = relu*e*rden*rS into L, with row-sum accumulation
                    nc.vector.tensor_scalar(
                        out=Lb[:, cs], in0=rel, scalar1=rS, scalar2=0.0,
                        op0=ALU.mult, op1=ALU.add,
                        accum_out=L14[:, c:c + 1])
                    # fp16 copy of RAW weights for mm2 (renorm folded into
                    # the mm2 output staging scale)
                    nc.scalar.copy(out=m16[:, cs], in_=Lb[:, cs])
                L1 = small.tile([128, 1], f32, tag="L1")
                nc.vector.reduce_sum(L1, L14, axis=AX.X)
                rL1 = rl1p.tile([128, 1], f32, tag="rL1")
                rL1s.append(rL1)
                nc.vector.reciprocal(rL1, L1)
                for c in range(NCH):
                    cs = slice(c * CH, (c + 1) * CH)
                    nc.vector.tensor_scalar_mul(Lb[:, cs], Lb[:, cs], rL1)
                nc.sync.dma_start(out=mw_d[bt * 128:(bt + 1) * 128, :], in_=Lb)
                # mem_weight^T tiles (fp16) for mm2 lhsT
                for tg in range(4):
                    pt = psB.tile([128, 512], f16, tag="ptr16")
                    for c in range(4):
                        mt = tg * 4 + c
                        mp = MT_SIZES[mt]
                        nc.tensor.transpose(
                            pt[:mp, c * 128:c * 128 + 128],
                            m16[:, MT_OFFS[mt]:MT_OFFS[mt] + mp],
                            ident16)
                    for c in range(4):
                        mt = tg * 4 + c
                        mp = MT_SIZES[mt]
                        eng = nc.vector.tensor_copy if c % 2 == 0 \
                            else nc.scalar.copy
                        eng(out=mwT[:mp, mt, bt * 128:(bt + 1) * 128],
                            in_=pt[:mp, c * 128:c * 128 + 128])

            # ---------------- Phase E: mm2 out = mem_weight @ w_hi --------
            NQ = 4
            DQ = D // NQ          # 1024 columns per quarter

            def load_q(q, w16_parts):
                for mt in range(NMT):
                    mp = MT_SIZES[mt]
                    moff = MT_OFFS[mt]
                    w16, mt0 = w16_parts[mt // 8] if len(w16_parts) == 2 \
                        else w16_parts[0]
                    nc.sync.dma_start(
                        out=w16[:mp, mt - mt0, :],
                        in_=whi_d[moff:moff + mp, q * DQ:(q + 1) * DQ])

            def mm2_q(q, w16_parts, bts=range(NBT)):
                for bt in bts:
                    for n in range(DQ // 512):
                        po = psA.tile([128, 512], f32,
                                      tag="pmm" if n == 0 else "pmmc")
                        for mt in range(NMT):
                            mp = MT_SIZES[mt]
                            w16, mt0 = w16_parts[mt // 8] \
                                if len(w16_parts) == 2 else w16_parts[0]
                            nc.tensor.matmul(
                                po, lhsT=mwT[:mp, mt, bt * 128:(bt + 1) * 128],
                                rhs=w16[:mp, mt - mt0, n * 512:(n + 1) * 512],
                                start=(mt == 0), stop=(mt == NMT - 1))
                        ost = stage.tile([128, 512], f32, tag="lt")
                        nc.scalar.mul(ost, po, rL1s[bt])
                        nc.sync.dma_start(
                            out=out_d[bt * 128:(bt + 1) * 128,
                                      q * DQ + n * 512:q * DQ + (n + 1) * 512],
                            in_=ost)

            # prefetch q0 + q1 as early as slots free (big8 after mm1,
            # io slots already free during the last mm1 tile)
            w16q0 = big8.tile([128, NMT, DQ], f16, tag="big")
            q0p = [(w16q0, 0)]
            load_q(0, q0p)
            w16q1a = io4k.tile([128, 8, DQ], f16, tag="io")
            w16q1b = io4k.tile([128, 8, DQ], f16, tag="io")
            q1p = [(w16q1a, 0), (w16q1b, 8)]
            load_q(1, q1p)
            for bt in range(NBT):
                mm2_q(0, q0p, bts=[bt])
                mm2_q(1, q1p, bts=[bt])
            w16q2 = lbuf.tile([128, NMT, DQ], f16, tag="L")
            q2p = [(w16q2, 0)]
            load_q(2, q2p)
            w16q3 = big8.tile([128, NMT, DQ], f16, tag="big")
            q3p = [(w16q3, 0)]
            load_q(3, q3p)
            mm2_q(2, q2p)
            mm2_q(3, q3p)
    nc.compile()
    return nc


def _get_nc():
    if "nc" not in _cache:
        _cache["nc"] = _build_nc()
    return _cache["nc"]


def kernel(x: np.ndarray, weight: np.ndarray):
    from concourse.bass_utils import run_bass_kernel_spmd

    x = np.ascontiguousarray(np.asarray(x, dtype=np.float32))
    weight = np.ascontiguousarray(np.asarray(weight, dtype=np.float32))
    nc = _get_nc()
    in_maps = [
        {"x": x[i * BS:(i + 1) * BS], "weight": weight} for i in range(NCORES)
    ]
    res = run_bass_kernel_spmd(nc, in_maps, core_ids=list(range(NCORES)))
    results = res.results
    output = np.concatenate([r["out"] for r in results], axis=0)
    mem_weight = np.concatenate([r["mem_weight"] for r in results], axis=0)
    return output, mem_weight


if __name__ == "__main__":
    xs = np.random.randn(B_FULL, D).astype(np.float32)
    ws = (np.random.randn(M, D) / np.sqrt(D)).astype(np.float32)
    o, mw = kernel(xs, ws)
    print(o.shape, mw.shape, o.dtype, mw.dtype)
